# revision 27
# baseline (speedup 1.0000x reference)
"""Trainium2 Bass kernel for a dense transformer block.

Reference computation (per batch element):
    y  = Attention(LN1(x)) ; x = x + y
    x  = x + MLP(LN2(x))
with B=8, N=1024, C=768, H=12 heads, head_dim=64, HIDDEN=3072, fp32 I/O.

Sharding: data-parallel over B across the 8 NeuronCores — each core runs the
full block on one (1024, 768) batch element with replicated weights. No
collectives.

Per-core design notes (v2 restructure):
  * Matmul operands are bf16; PSUM accumulation and the residual stream /
    layernorm statistics stay fp32.
  * Dense matmuls amortize LDWEIGHTS over two moving-operand halves by
    keeping one stationary operand for both output column blocks (two PSUM
    banks accumulate concurrently).
  * Attention is head-pipelined: S^T matmuls for head h fill a 2-bank PSUM
    tile (one key tile x 1024 query cols), which a single [128,1024] Exp
    evicts to an es tile; AV matmuls of head h-1 interleave on the PE while
    the ACT engine streams exps. Softmax denominators ride along as a
    ones-column appended to V (row 64 of the AV accumulator).
  * Softmax normalization: the denominator row bounces through DRAM into a
    [128, 8] compact layout for the (slow, iterative) DVE reciprocal, then
    broadcasts back over 64 partitions; the normalize multiply reads the AV
    PSUM directly and writes attnT (even heads in place, odd heads via an
    SBUF bounce + DMA, since DVE lanes cannot shift partitions).
  * wproj/wfc1/wfc2 are DMA-prefetched during attention so the MLP never
    waits on HBM.
"""

import numpy as np
import ml_dtypes

import concourse.bass as bass
import concourse.bacc as bacc
import concourse.mybir as mybir
import concourse.tile as tile
from concourse import bass_utils

# Model dims (hardcoded per the problem spec).
B = 8
N = 1024  # tokens
C = 768  # model dim
H = 12  # heads
HD = 64  # head dim
HID = 3072  # mlp hidden
EPS = 1e-5
P = 128  # SBUF partitions

NT = N // P  # 8 token tiles
KC = C // P  # 6 contraction tiles over C
KH = HID // P  # 24 contraction tiles over HIDDEN

F32 = mybir.dt.float32
BF16 = mybir.dt.bfloat16
AF = mybir.ActivationFunctionType
ALU = mybir.AluOpType

_cache = {}

# CoreSim doesn't implement the Gelu activation table; when True the builder
# emits a tanh-approximation decomposition instead (dev/validation only).
SIM_GELU = False


def _build(flags):
    """Trace the per-core Bass program. `flags` gates optional bias/gain work."""
    (use_bqkv, use_g1, use_beta1, use_g2, use_beta2, use_bfc1, use_bproj,
     use_bfc2) = flags

    nc = bacc.Bacc("TRN2", target_bir_lowering=False, debug=False)

    x_d = nc.dram_tensor("x", [N, C], F32, kind="ExternalInput")
    wqkv_d = nc.dram_tensor("wqkv", [C, 3 * C], BF16, kind="ExternalInput")
    wproj_d = nc.dram_tensor("wproj", [C, C], BF16, kind="ExternalInput")
    wfc1_d = nc.dram_tensor("wfc1", [C, HID], BF16, kind="ExternalInput")
    wfc2_d = nc.dram_tensor("wfc2", [HID, C], BF16, kind="ExternalInput")
    out_d = nc.dram_tensor("out", [N, C], F32, kind="ExternalOutput")

    opt_d = {}
    for name, use, shape in (
        ("bqkv", use_bqkv, [3 * C]),
        ("g1", use_g1, [C]),
        ("beta1", use_beta1, [C]),
        ("g2", use_g2, [C]),
        ("beta2", use_beta2, [C]),
        ("bfc1", use_bfc1, [HID]),
        ("bproj", use_bproj, [C]),
        ("bfc2", use_bfc2, [C]),
    ):
        if use:
            opt_d[name] = nc.dram_tensor(name, shape, F32, kind="ExternalInput")

    def bcast_from_dram(pool, ap_1d, n):
        """[n] DRAM vector -> [P, n] SBUF tile replicated on every partition."""
        t = pool.tile([P, n], F32, name=f"bc_{ap_1d.tensor.name}")
        src = bass.AP(tensor=ap_1d.tensor, offset=ap_1d.offset,
                      ap=[[0, P]] + list(ap_1d.ap))
        nc.sync.dma_start(out=t, in_=src)
        return t

    with tile.TileContext(nc) as tc:
        persist = tc.alloc_tile_pool(name="persist", bufs=1, side="left")
        psum = tc.alloc_tile_pool(name="psum", bufs=1, space="PSUM")
        dram = tc.alloc_tile_pool(name="dram", bufs=2, space="DRAM")

        # Residual stream, token-major; updated in place through the block.
        x_sb = persist.tile([P, NT, C], F32)
        nc.sync.dma_start(out=x_sb[:, 0, :], in_=x_d.ap()[0:P, :])
        eps_t = persist.tile([P, 1], F32)
        nc.vector.memset(eps_t, EPS)

        # Identity (bf16, embedded in the NEFF) for PE-based transposes.
        ident_d = nc.inline_tensor(np.eye(P, dtype=ml_dtypes.bfloat16), "ident")
        ident = persist.tile([P, P], BF16)
        nc.sync.dma_start(out=ident, in_=ident_d.ap())

        g_beta = {}
        for name, n in (("g1", C), ("beta1", C), ("g2", C), ("beta2", C),
                        ("bproj", C), ("bfc2", C)):
            if name in opt_d:
                g_beta[name] = bcast_from_dram(persist, opt_d[name].ap(), n)
        bqkv_sb = None
        if "bqkv" in opt_d:
            bqkv_sb = persist.tile([P, 3 * C // P], F32)
            nc.sync.dma_start(out=bqkv_sb,
                              in_=opt_d["bqkv"].ap().rearrange("(m p) -> p m", p=P))
        bfc1_sb = None
        if "bfc1" in opt_d:
            bfc1_sb = persist.tile([P, KH], F32)
            nc.sync.dma_start(out=bfc1_sb,
                              in_=opt_d["bfc1"].ap().rearrange("(m p) -> p m", p=P))

        # ---------------------------------------------------------------
        # Phase 1: LN1 (token-major) -> x_lnT (feature-major bf16), weights
        # ---------------------------------------------------------------
        p1 = tc.alloc_tile_pool(name="p1", bufs=1, side="left")
        ln1 = tc.alloc_tile_pool(name="ln1", bufs=3, side="left")

        wqkv_sb = p1.tile([P, KC, 3 * C], BF16)
        for t in range(1, NT):
            nc.sync.dma_start(out=x_sb[:, t, :], in_=x_d.ap()[t * P:(t + 1) * P, :])
        for ko in range(KC):
            nc.sync.dma_start(out=wqkv_sb[:, ko, :],
                              in_=wqkv_d.ap()[ko * P:(ko + 1) * P, :])

        xlnT = p1.tile([P, KC, N], BF16)

        def layernorm_tile(pool, x_ap, g_sb, beta_sb, name):
            """x_ap: [P, C] fp32 token-major -> returns [P, C] bf16 tile."""
            stats = pool.tile([P, 3, 6], F32, tag=f"{name}_st", bufs=3)
            xr = x_ap.rearrange("p (s f) -> p s f", f=256)
            for s in range(3):
                nc.vector.bn_stats(out=stats[:, s, :], in_=xr[:, s, :])
            mv = pool.tile([P, 2], F32, tag=f"{name}_mv", bufs=3)
            nc.vector.bn_aggr(out=mv, in_=stats)
            rstd = pool.tile([P, 1], F32, tag=f"{name}_rs", bufs=3)
            nc.scalar.activation(out=rstd, in_=mv[:, 1:2], func=AF.Sqrt,
                                 bias=eps_t, scale=1.0)
            nc.vector.reciprocal(out=rstd, in_=rstd)
            xln = pool.tile([P, C], BF16, tag=f"{name}_xln", bufs=3)
            nc.vector.tensor_scalar(out=xln, in0=x_ap, scalar1=mv[:, 0:1],
                                    scalar2=rstd, op0=ALU.subtract, op1=ALU.mult)
            if g_sb is not None:
                nc.vector.tensor_mul(out=xln, in0=xln, in1=g_sb)
            if beta_sb is not None:
                nc.vector.tensor_add(out=xln, in0=xln, in1=beta_sb)
            return xln

        def transpose_to(xln, dstT, t):
            """[P, C] token-major tile -> dstT[:, :, t*P:(t+1)*P] feature-major.

            Uses the DMA XBAR transpose (16-bit, SBUF-to-SBUF) — no PE array
            time, no PSUM traffic, no ACT eviction copies.
            """
            for c in range(KC):
                nc.sync.dma_start(out=dstT[:, c, t * P:(t + 1) * P],
                                  in_=xln[:, c * P:(c + 1) * P], transpose=True)

        for t in range(NT):
            xln = layernorm_tile(ln1, x_sb[:, t, :], g_beta.get("g1"),
                                 g_beta.get("beta1"), "ln1")
            transpose_to(xln, xlnT, t)

        # ---------------------------------------------------------------
        # Phase 2: qkv projections.
        #   q^T,k^T feature-major: [2C, N] as 12 tiles of [128, N]
        #   V token-major with ones column: V_aug [P, NT, H, HD+1]
        # ---------------------------------------------------------------
        p2 = tc.alloc_tile_pool(name="p2", bufs=1, side="right")
        qkT = p2.tile([P, 2 * KC, N], BF16)
        # ones column appended to V: the AV matmul's row 64 is then the
        # per-query sum of exp-scores (softmax denominator). V values sit on
        # partitions 0..63 (a >32-partition DVE access must start at 0/64).
        VW = HD + 1
        v_aug = p2.tile([P, NT, H, VW], BF16)
        nc.vector.memset(v_aug[:, :, :, HD:HD + 1], 1.0)

        # q^T / k^T: out^T[m-block, tokens] = wqkv[:, m-block].T @ x_ln^T
        # Loop i-outer so head pair i's q AND k complete together (the
        # attention pipeline starts as soon as pair 0 is done); per (i, m)
        # one stationary operand serves both 512-column moving halves.
        for i in range(KC):
            for m in (i, KC + i):
                ps = psum.tile([P, 2, 512], F32, tag="sps", bufs=2, name="ps_qk")
                for ko in range(KC):
                    for j in range(2):
                        nc.tensor.matmul(ps[:, j, :],
                                         wqkv_sb[:, ko, m * P:(m + 1) * P],
                                         xlnT[:, ko, j * 512:(j + 1) * 512],
                                         start=(ko == 0), stop=(ko == KC - 1))
                for j in range(2):
                    if bqkv_sb is not None:
                        nc.any.tensor_scalar_add(qkT[:, m, j * 512:(j + 1) * 512],
                                                 ps[:, j, :], bqkv_sb[:, m:m + 1])
                    else:
                        nc.vector.tensor_copy(out=qkT[:, m, j * 512:(j + 1) * 512],
                                              in_=ps[:, j, :])

        # V token-major: V[tok-tile, vfeats] = x_ln @ wqkv[:, 2C:3C]
        bqv = None
        if bqkv_sb is not None:
            bqv = bcast_from_dram(persist, opt_d["bqkv"].ap()[2 * C:3 * C], C)
        for t in range(NT):
            ps = psum.tile([P, 2, 512], F32, tag="sps", bufs=2, name="ps_v")
            for ko in range(KC):
                for j, (n0, nn) in enumerate(((0, 512), (512, 256))):
                    nc.tensor.matmul(ps[:, j, :nn],
                                     xlnT[:, ko, t * P:(t + 1) * P],
                                     wqkv_sb[:, ko, 2 * C + n0:2 * C + n0 + nn],
                                     start=(ko == 0), stop=(ko == KC - 1))
            for j, (n0, nn) in enumerate(((0, 512), (512, 256))):
                nh = nn // HD
                dst = v_aug[:, t, j * 8:j * 8 + nh, 0:HD]
                src = ps[:, j, :nn].rearrange("p (h d) -> p h d", d=HD)
                if bqv is not None:
                    nc.any.tensor_add(out=dst, in0=src,
                                      in1=bqv[:, n0:n0 + nn].rearrange(
                                          "p (h d) -> p h d", d=HD))
                else:
                    nc.vector.tensor_copy(out=dst, in_=src)

        ln1.release()
        p1.release()

        # ---------------------------------------------------------------
        # Prefetch the remaining weights during attention (SBUF freed by p1).
        # ---------------------------------------------------------------
        wpre = tc.alloc_tile_pool(name="wpre", bufs=1, side="left")
        wproj_sb = wpre.tile([P, KC, C], BF16)
        wfc1_sb = wpre.tile([P, KC, HID], BF16)
        for ko in range(KC):
            nc.sync.dma_start(out=wproj_sb[:, ko, :],
                              in_=wproj_d.ap()[ko * P:(ko + 1) * P, :])
        for ko in range(KC):
            nc.sync.dma_start(out=wfc1_sb[:, ko, :],
                              in_=wfc1_d.ap()[ko * P:(ko + 1) * P, :])

        # ---------------------------------------------------------------
        # Phase 3: attention, head-pipelined (S/exp of head h overlaps AV of
        # head h-1 on the PE; ACT streams one [128,1024] exp per key tile).
        # ---------------------------------------------------------------
        p3 = tc.alloc_tile_pool(name="p3", bufs=1, side="left")
        att = tc.alloc_tile_pool(name="att", bufs=1, side="right")
        attnT = p3.tile([P, KC, N], BF16)

        def s_chunk(h, es_h, kt):
            pb = (h % 2) * HD
            qT = qkT[pb:pb + HD, h // 2, :]
            kT = qkT[pb:pb + HD, KC + h // 2, :]
            sps = psum.tile([P, 2, 512], F32, tag="sps", bufs=2,
                            name=f"s_{h}_{kt}")
            for j in range(2):
                nc.tensor.matmul(sps[:, j, :], kT[:, kt * P:(kt + 1) * P],
                                 qT[:, j * 512:(j + 1) * 512],
                                 start=True, stop=True)
            nc.scalar.activation(out=es_h[:, kt, :],
                                 in_=sps.rearrange("p a b -> p (a b)"),
                                 func=AF.Exp, scale=0.125)

        def av_chunk(h, es_h, avs, kt):
            for j in range(2):
                nc.tensor.matmul(avs[j], v_aug[:, kt, h, :],
                                 es_h[:, kt, j * 512:(j + 1) * 512],
                                 start=(kt == 0), stop=(kt == NT - 1))

        def norm_head(h, avs):
            pb = (h % 2) * HD
            # Softmax denominators: row HD holds sum_k exp(S). Copy it out on
            # its own lane (partition 64), bounce through DRAM into a
            # [128, 8] compact layout so the (slow, per-lane iterative)
            # reciprocal runs across all partitions, then broadcast back over
            # partitions 0..63 (step-0 partition reads are only legal from
            # DRAM).
            den = att.tile([VW, N], F32, tag="den", bufs=2, name=f"den{h}")
            for j in range(2):
                nc.vector.tensor_copy(out=den[HD:VW, j * 512:(j + 1) * 512],
                                      in_=avs[j][HD:VW, :])
            rcomp = att.tile([P, NT], F32, tag="rc", bufs=2, name=f"rc{h}")
            nc.sync.dma_start(out=rcomp, in_=den[HD:VW, :])
            nc.vector.reciprocal(out=rcomp, in_=rcomp)
            rd2 = dram.tile([1, N], F32, tag="rd2", bufs=3)
            nc.sync.dma_start(out=rd2[0, :].rearrange("(p m) -> p m", p=P),
                              in_=rcomp)
            rbc = att.tile([HD, N], F32, tag="rbc", bufs=2, name=f"rbc{h}")
            rd2a = rd2[0, :]
            rbc_src = bass.AP(tensor=rd2a.tensor, offset=rd2a.offset,
                              ap=[[0, HD]] + list(rd2a.ap))
            nc.sync.dma_start(out=rbc, in_=rbc_src)
            # Normalize straight out of PSUM. Even heads write attnT's
            # partitions 0..63 directly; odd heads bounce via SBUF + DMA
            # (DVE lanes cannot shift partition ranges).
            if pb == 0:
                for j in range(2):
                    nc.vector.tensor_mul(
                        out=attnT[0:HD, h // 2, j * 512:(j + 1) * 512],
                        in0=avs[j][0:HD, :], in1=rbc[:, j * 512:(j + 1) * 512])
            else:
                bounce = att.tile([HD, N], BF16, tag="bounce", bufs=2,
                                  name=f"bounce{h}")
                for j in range(2):
                    nc.vector.tensor_mul(out=bounce[:, j * 512:(j + 1) * 512],
                                         in0=avs[j][0:HD, :],
                                         in1=rbc[:, j * 512:(j + 1) * 512])
                nc.sync.dma_start(out=attnT[pb:pb + HD, h // 2, :], in_=bounce)

        # Per head: S/exp chunks stream per key tile with the head's own AV
        # chunks interleaved two key tiles behind — the PE alternates S and
        # AV work at fine grain so the ACT exp stream (the attention-phase
        # bottleneck) never starves. Pair-swapped order makes the final head
        # even, whose normalize writes attnT directly (shortest tail).
        order = [1, 0, 3, 2, 5, 4, 7, 6, 9, 8, 11, 10]
        for h in order:
            es_h = att.tile([P, NT, N], BF16, tag="es", bufs=2, name=f"es{h}")
            avs = [psum.tile([VW, 512], F32, tag="av", bufs=4,
                             name=f"av{j}_{h}") for j in range(2)]
            for kt in range(NT):
                s_chunk(h, es_h, kt)
                if kt >= 2:
                    av_chunk(h, es_h, avs, kt - 2)
            for kt in range(NT - 2, NT):
                av_chunk(h, es_h, avs, kt)
            norm_head(h, avs)

        att.release()
        p2.release()

        # ---------------------------------------------------------------
        # Phase 4: proj + residual, LN2 -> x2_lnT
        # ---------------------------------------------------------------
        # wfc2 isn't needed until phase 6; loading it here (into SBUF freed
        # by the attention pools) keeps the attention-phase footprint low.
        w2 = tc.alloc_tile_pool(name="w2", bufs=1, side="right")
        wfc2_sb = w2.tile([P, KH, C], BF16)
        for ko in range(KH):
            nc.sync.dma_start(out=wfc2_sb[:, ko, :],
                              in_=wfc2_d.ap()[ko * P:(ko + 1) * P, :])

        p4 = tc.alloc_tile_pool(name="p4", bufs=1, side="right")
        ln2 = tc.alloc_tile_pool(name="ln2", bufs=3, side="right")
        x2lnT = p4.tile([P, KC, N], BF16)

        for t in range(NT):
            ps = psum.tile([P, 2, 512], F32, tag="sps", bufs=2, name="ps_pr")
            for ko in range(KC):
                for j, (n0, nn) in enumerate(((0, 512), (512, 256))):
                    nc.tensor.matmul(ps[:, j, :nn],
                                     attnT[:, ko, t * P:(t + 1) * P],
                                     wproj_sb[:, ko, n0:n0 + nn],
                                     start=(ko == 0), stop=(ko == KC - 1))
            for j, (n0, nn) in enumerate(((0, 512), (512, 256))):
                xs = x_sb[:, t, n0:n0 + nn]
                nc.vector.tensor_add(out=xs, in0=xs, in1=ps[:, j, :nn])
                if "bproj" in g_beta:
                    nc.vector.tensor_add(out=xs, in0=xs,
                                         in1=g_beta["bproj"][:, n0:n0 + nn])
            xln = layernorm_tile(ln2, x_sb[:, t, :], g_beta.get("g2"),
                                 g_beta.get("beta2"), "ln2")
            transpose_to(xln, x2lnT, t)

        p3.release()

        # ---------------------------------------------------------------
        # Phase 5: fc1 + gelu -> h^T (feature-major bf16)
        # ---------------------------------------------------------------
        p5 = tc.alloc_tile_pool(name="p5", bufs=1, side="left")
        hT = p5.tile([P, KH, N], BF16)

        for m in range(KH):
            ps = psum.tile([P, 2, 512], F32, tag="sps", bufs=2, name="ps_f1")
            for ko in range(KC):
                for j in range(2):
                    nc.tensor.matmul(ps[:, j, :],
                                     wfc1_sb[:, ko, m * P:(m + 1) * P],
                                     x2lnT[:, ko, j * 512:(j + 1) * 512],
                                     start=(ko == 0), stop=(ko == KC - 1))
            bias = bfc1_sb[:, m:m + 1] if bfc1_sb is not None else 0.0
            for j in range(2):
                if not SIM_GELU:
                    nc.scalar.activation(out=hT[:, m, j * 512:(j + 1) * 512],
                                         in_=ps[:, j, :], func=AF.Gelu,
                                         bias=bias, scale=1.0)
                else:
                    a = ln2.tile([P, 512], F32, tag="g_a", bufs=2)
                    nc.scalar.activation(out=a, in_=ps[:, j, :], func=AF.Copy,
                                         bias=0.0, scale=1.0)
                    if bfc1_sb is not None:
                        nc.vector.tensor_scalar_add(a, a, bfc1_sb[:, m:m + 1])
                    u = ln2.tile([P, 512], F32, tag="g_u", bufs=2)
                    nc.vector.tensor_mul(out=u, in0=a, in1=a)
                    nc.vector.tensor_mul(out=u, in0=u, in1=a)
                    nc.vector.tensor_scalar_mul(u, u, 0.044715)
                    nc.vector.tensor_add(out=u, in0=u, in1=a)
                    nc.scalar.activation(out=u, in_=u, func=AF.Tanh,
                                         bias=0.0, scale=0.7978845608028654)
                    nc.vector.tensor_scalar_add(u, u, 1.0)
                    nc.vector.tensor_scalar_mul(a, a, 0.5)
                    nc.vector.tensor_mul(out=hT[:, m, j * 512:(j + 1) * 512],
                                         in0=a, in1=u)

        ln2.release()
        p4.release()

        # ---------------------------------------------------------------
        # Phase 6: fc2 + residual -> out
        # ---------------------------------------------------------------
        for t in range(NT):
            ps = psum.tile([P, 2, 512], F32, tag="sps", bufs=2, name="ps_f2")
            for ko in range(KH):
                for j, (n0, nn) in enumerate(((0, 512), (512, 256))):
                    nc.tensor.matmul(ps[:, j, :nn],
                                     hT[:, ko, t * P:(t + 1) * P],
                                     wfc2_sb[:, ko, n0:n0 + nn],
                                     start=(ko == 0), stop=(ko == KH - 1))
            for j, (n0, nn) in enumerate(((0, 512), (512, 256))):
                xs = x_sb[:, t, n0:n0 + nn]
                nc.vector.tensor_add(out=xs, in0=xs, in1=ps[:, j, :nn])
                if "bfc2" in g_beta:
                    nc.vector.tensor_add(out=xs, in0=xs,
                                         in1=g_beta["bfc2"][:, n0:n0 + nn])
            nc.sync.dma_start(out=out_d.ap()[t * P:(t + 1) * P, :],
                              in_=x_sb[:, t, :])

        w2.release()
        p5.release()
        wpre.release()
        persist.release()
        dram.release()
        psum.release()

    nc.compile()
    return nc


def _prep(inputs):
    """Host-side prep: shard x over B, cast weights to bf16, compute gates."""
    f = {k: np.asarray(v) for k, v in inputs.items()}
    bf = ml_dtypes.bfloat16

    flags = (
        bool(np.any(f["b_qkv"])),
        not np.all(f["g1"] == 1.0),
        bool(np.any(f["beta1"])),
        not np.all(f["g2"] == 1.0),
        bool(np.any(f["beta2"])),
        bool(np.any(f["b_fc1"])),
        bool(np.any(f["b_proj"])),
        bool(np.any(f["b_fc2"])),
    )
    (use_bqkv, use_g1, use_beta1, use_g2, use_beta2, use_bfc1, use_bproj,
     use_bfc2) = flags

    common = {
        "wqkv": np.ascontiguousarray(f["w_qkv"].astype(bf)),
        "wproj": np.ascontiguousarray(f["w_proj"].astype(bf)),
        "wfc1": np.ascontiguousarray(f["w_fc1"].astype(bf)),
        "wfc2": np.ascontiguousarray(f["w_fc2"].astype(bf)),
    }
    for name, key, use in (
        ("bqkv", "b_qkv", use_bqkv), ("g1", "g1", use_g1),
        ("beta1", "beta1", use_beta1), ("g2", "g2", use_g2),
        ("beta2", "beta2", use_beta2), ("bfc1", "b_fc1", use_bfc1),
        ("bproj", "b_proj", use_bproj), ("bfc2", "b_fc2", use_bfc2),
    ):
        if use:
            common[name] = np.ascontiguousarray(f[key].astype(np.float32))

    x = f["x"].astype(np.float32)
    in_maps = [dict(common, x=np.ascontiguousarray(x[i])) for i in range(B)]
    return flags, in_maps


LAST_RESULT = None


def kernel(**inputs):
    global LAST_RESULT
    flags, in_maps = _prep(inputs)
    if flags not in _cache:
        _cache[flags] = _build(flags)
    nc = _cache[flags]
    res = bass_utils.run_bass_kernel_spmd(nc, in_maps, core_ids=list(range(B)))
    LAST_RESULT = res
    out = np.stack([r["out"] for r in res.results], axis=0)
    return out.astype(np.float32)


# revision 28
# speedup vs baseline: 1.2419x; 1.2419x over previous
"""Trainium2 Bass kernel for a dense transformer block.

Reference computation (per batch element):
    y  = Attention(LN1(x)) ; x = x + y
    x  = x + MLP(LN2(x))
with B=8, N=1024, C=768, H=12 heads, head_dim=64, HIDDEN=3072, fp32 I/O.

Sharding: data-parallel over B across the 8 NeuronCores — each core runs the
full block on one (1024, 768) batch element with replicated weights. No
collectives.

Per-core design notes (v2 restructure):
  * Matmul operands are bf16; PSUM accumulation and the residual stream /
    layernorm statistics stay fp32.
  * Dense matmuls amortize LDWEIGHTS over two moving-operand halves by
    keeping one stationary operand for both output column blocks (two PSUM
    banks accumulate concurrently).
  * Attention is head-pipelined: S^T matmuls for head h fill a 2-bank PSUM
    tile (one key tile x 1024 query cols), which a single [128,1024] Exp
    evicts to an es tile; AV matmuls of head h-1 interleave on the PE while
    the ACT engine streams exps. Softmax denominators ride along as a
    ones-column appended to V (row 64 of the AV accumulator).
  * Softmax normalization: the denominator row bounces through DRAM into a
    [128, 8] compact layout for the (slow, iterative) DVE reciprocal, then
    broadcasts back over 64 partitions; the normalize multiply reads the AV
    PSUM directly and writes attnT (even heads in place, odd heads via an
    SBUF bounce + DMA, since DVE lanes cannot shift partitions).
  * wproj/wfc1/wfc2 are DMA-prefetched during attention so the MLP never
    waits on HBM.
"""

import numpy as np
import ml_dtypes

import concourse.bass as bass
import concourse.bacc as bacc
import concourse.mybir as mybir
import concourse.tile as tile
from concourse import bass_utils

# Model dims (hardcoded per the problem spec).
B = 8
N = 1024  # tokens
C = 768  # model dim
H = 12  # heads
HD = 64  # head dim
HID = 3072  # mlp hidden
EPS = 1e-5
P = 128  # SBUF partitions

NT = N // P  # 8 token tiles
KC = C // P  # 6 contraction tiles over C
KH = HID // P  # 24 contraction tiles over HIDDEN

F32 = mybir.dt.float32
BF16 = mybir.dt.bfloat16
AF = mybir.ActivationFunctionType
ALU = mybir.AluOpType

_cache = {}

# CoreSim doesn't implement the Gelu activation table; when True the builder
# emits a tanh-approximation decomposition instead (dev/validation only).
SIM_GELU = False


def _build(flags):
    """Trace the per-core Bass program. `flags` gates optional bias/gain work."""
    (use_bqkv, use_g1, use_beta1, use_g2, use_beta2, use_bfc1, use_bproj,
     use_bfc2) = flags

    nc = bacc.Bacc("TRN2", target_bir_lowering=False, debug=False)

    x_d = nc.dram_tensor("x", [N, C], F32, kind="ExternalInput")
    wqkv_d = nc.dram_tensor("wqkv", [C, 3 * C], BF16, kind="ExternalInput")
    wproj_d = nc.dram_tensor("wproj", [C, C], BF16, kind="ExternalInput")
    wfc1_d = nc.dram_tensor("wfc1", [C, HID], BF16, kind="ExternalInput")
    wfc2_d = nc.dram_tensor("wfc2", [HID, C], BF16, kind="ExternalInput")
    out_d = nc.dram_tensor("out", [N, C], F32, kind="ExternalOutput")

    opt_d = {}
    for name, use, shape in (
        ("bqkv", use_bqkv, [3 * C]),
        ("g1", use_g1, [C]),
        ("beta1", use_beta1, [C]),
        ("g2", use_g2, [C]),
        ("beta2", use_beta2, [C]),
        ("bfc1", use_bfc1, [HID]),
        ("bproj", use_bproj, [C]),
        ("bfc2", use_bfc2, [C]),
    ):
        if use:
            opt_d[name] = nc.dram_tensor(name, shape, F32, kind="ExternalInput")

    def bcast_from_dram(pool, ap_1d, n):
        """[n] DRAM vector -> [P, n] SBUF tile replicated on every partition."""
        t = pool.tile([P, n], F32, name=f"bc_{ap_1d.tensor.name}")
        src = bass.AP(tensor=ap_1d.tensor, offset=ap_1d.offset,
                      ap=[[0, P]] + list(ap_1d.ap))
        nc.sync.dma_start(out=t, in_=src)
        return t

    with tile.TileContext(nc) as tc:
        persist = tc.alloc_tile_pool(name="persist", bufs=1, side="left")
        psum = tc.alloc_tile_pool(name="psum", bufs=1, space="PSUM")
        dram = tc.alloc_tile_pool(name="dram", bufs=2, space="DRAM")

        # Residual stream, token-major; updated in place through the block.
        x_sb = persist.tile([P, NT, C], F32)
        nc.sync.dma_start(out=x_sb[:, 0, :], in_=x_d.ap()[0:P, :])
        eps_t = persist.tile([P, 1], F32)
        nc.vector.memset(eps_t, EPS)

        # Identity (bf16, embedded in the NEFF) for PE-based transposes.
        ident_d = nc.inline_tensor(np.eye(P, dtype=ml_dtypes.bfloat16), "ident")
        ident = persist.tile([P, P], BF16)
        nc.sync.dma_start(out=ident, in_=ident_d.ap())

        g_beta = {}
        for name, n in (("g1", C), ("beta1", C), ("g2", C), ("beta2", C),
                        ("bproj", C), ("bfc2", C)):
            if name in opt_d:
                g_beta[name] = bcast_from_dram(persist, opt_d[name].ap(), n)
        bqkv_sb = None
        if "bqkv" in opt_d:
            bqkv_sb = persist.tile([P, 3 * C // P], F32)
            nc.sync.dma_start(out=bqkv_sb,
                              in_=opt_d["bqkv"].ap().rearrange("(m p) -> p m", p=P))
        bfc1_sb = None
        if "bfc1" in opt_d:
            bfc1_sb = persist.tile([P, KH], F32)
            nc.sync.dma_start(out=bfc1_sb,
                              in_=opt_d["bfc1"].ap().rearrange("(m p) -> p m", p=P))

        # ---------------------------------------------------------------
        # Phase 1: LN1 (token-major) -> x_lnT (feature-major bf16), weights
        # ---------------------------------------------------------------
        p1 = tc.alloc_tile_pool(name="p1", bufs=1, side="left")
        ln1 = tc.alloc_tile_pool(name="ln1", bufs=3, side="left")

        wqkv_sb = p1.tile([P, KC, 3 * C], BF16)
        for t in range(1, NT):
            nc.sync.dma_start(out=x_sb[:, t, :], in_=x_d.ap()[t * P:(t + 1) * P, :])
        for ko in range(KC):
            nc.sync.dma_start(out=wqkv_sb[:, ko, :],
                              in_=wqkv_d.ap()[ko * P:(ko + 1) * P, :])

        xlnT = p1.tile([P, KC, N], BF16)

        def layernorm_tile(pool, x_ap, g_sb, beta_sb, name):
            """x_ap: [P, C] fp32 token-major -> returns [P, C] bf16 tile."""
            stats = pool.tile([P, 3, 6], F32, tag=f"{name}_st", bufs=3)
            xr = x_ap.rearrange("p (s f) -> p s f", f=256)
            for s in range(3):
                nc.vector.bn_stats(out=stats[:, s, :], in_=xr[:, s, :])
            mv = pool.tile([P, 2], F32, tag=f"{name}_mv", bufs=3)
            nc.vector.bn_aggr(out=mv, in_=stats)
            rstd = pool.tile([P, 1], F32, tag=f"{name}_rs", bufs=3)
            nc.scalar.activation(out=rstd, in_=mv[:, 1:2], func=AF.Sqrt,
                                 bias=eps_t, scale=1.0)
            nc.vector.reciprocal(out=rstd, in_=rstd)
            xln = pool.tile([P, C], BF16, tag=f"{name}_xln", bufs=3)
            nc.vector.tensor_scalar(out=xln, in0=x_ap, scalar1=mv[:, 0:1],
                                    scalar2=rstd, op0=ALU.subtract, op1=ALU.mult)
            if g_sb is not None:
                nc.vector.tensor_mul(out=xln, in0=xln, in1=g_sb)
            if beta_sb is not None:
                nc.vector.tensor_add(out=xln, in0=xln, in1=beta_sb)
            return xln

        def transpose_to(xln, dstT, t):
            """[P, C] token-major tile -> dstT[:, :, t*P:(t+1)*P] feature-major."""
            for c in range(KC):
                tps = psum.tile([P, P], BF16, tag="av", bufs=4, name="tps")
                nc.tensor.transpose(tps, xln[:, c * P:(c + 1) * P], ident)
                nc.scalar.copy(out=dstT[:, c, t * P:(t + 1) * P], in_=tps)

        for t in range(NT):
            xln = layernorm_tile(ln1, x_sb[:, t, :], g_beta.get("g1"),
                                 g_beta.get("beta1"), "ln1")
            transpose_to(xln, xlnT, t)

        # ---------------------------------------------------------------
        # Phase 2: qkv projections.
        #   q^T,k^T feature-major: [2C, N] as 12 tiles of [128, N]
        #   V token-major with ones column: V_aug [P, NT, H, HD+1]
        # ---------------------------------------------------------------
        p2 = tc.alloc_tile_pool(name="p2", bufs=1, side="right")
        qkT = p2.tile([P, 2 * KC, N], BF16)
        # ones column appended to V: the AV matmul's row 64 is then the
        # per-query sum of exp-scores (softmax denominator). V values sit on
        # partitions 0..63 (a >32-partition DVE access must start at 0/64).
        VW = HD + 1
        v_aug = p2.tile([P, NT, H, VW], BF16)
        nc.vector.memset(v_aug[:, :, :, HD:HD + 1], 1.0)

        # q^T / k^T: out^T[m-block, tokens] = wqkv[:, m-block].T @ x_ln^T
        # Loop i-outer so head pair i's q AND k complete together (the
        # attention pipeline starts as soon as pair 0 is done); per (i, m)
        # one stationary operand serves both 512-column moving halves.
        for i in range(KC):
            for m in (i, KC + i):
                ps = psum.tile([P, 2, 512], F32, tag="sps", bufs=2, name="ps_qk")
                for ko in range(KC):
                    for j in range(2):
                        nc.tensor.matmul(ps[:, j, :],
                                         wqkv_sb[:, ko, m * P:(m + 1) * P],
                                         xlnT[:, ko, j * 512:(j + 1) * 512],
                                         start=(ko == 0), stop=(ko == KC - 1))
                for j in range(2):
                    if bqkv_sb is not None:
                        nc.any.tensor_scalar_add(qkT[:, m, j * 512:(j + 1) * 512],
                                                 ps[:, j, :], bqkv_sb[:, m:m + 1])
                    else:
                        nc.vector.tensor_copy(out=qkT[:, m, j * 512:(j + 1) * 512],
                                              in_=ps[:, j, :])

        # V token-major: V[tok-tile, vfeats] = x_ln @ wqkv[:, 2C:3C]
        bqv = None
        if bqkv_sb is not None:
            bqv = bcast_from_dram(persist, opt_d["bqkv"].ap()[2 * C:3 * C], C)
        for t in range(NT):
            ps = psum.tile([P, 2, 512], F32, tag="sps", bufs=2, name="ps_v")
            for ko in range(KC):
                for j, (n0, nn) in enumerate(((0, 512), (512, 256))):
                    nc.tensor.matmul(ps[:, j, :nn],
                                     xlnT[:, ko, t * P:(t + 1) * P],
                                     wqkv_sb[:, ko, 2 * C + n0:2 * C + n0 + nn],
                                     start=(ko == 0), stop=(ko == KC - 1))
            for j, (n0, nn) in enumerate(((0, 512), (512, 256))):
                nh = nn // HD
                dst = v_aug[:, t, j * 8:j * 8 + nh, 0:HD]
                src = ps[:, j, :nn].rearrange("p (h d) -> p h d", d=HD)
                if bqv is not None:
                    nc.any.tensor_add(out=dst, in0=src,
                                      in1=bqv[:, n0:n0 + nn].rearrange(
                                          "p (h d) -> p h d", d=HD))
                else:
                    nc.vector.tensor_copy(out=dst, in_=src)

        ln1.release()
        p1.release()

        # ---------------------------------------------------------------
        # Prefetch the remaining weights during attention (SBUF freed by p1).
        # ---------------------------------------------------------------
        wpre = tc.alloc_tile_pool(name="wpre", bufs=1, side="left")
        wproj_sb = wpre.tile([P, KC, C], BF16)
        wfc1_sb = wpre.tile([P, KC, HID], BF16)
        for ko in range(KC):
            nc.sync.dma_start(out=wproj_sb[:, ko, :],
                              in_=wproj_d.ap()[ko * P:(ko + 1) * P, :])
        for ko in range(KC):
            nc.sync.dma_start(out=wfc1_sb[:, ko, :],
                              in_=wfc1_d.ap()[ko * P:(ko + 1) * P, :])

        # ---------------------------------------------------------------
        # Phase 3: attention, head-pipelined (S/exp of head h overlaps AV of
        # head h-1 on the PE; ACT streams one [128,1024] exp per key tile).
        # ---------------------------------------------------------------
        p3 = tc.alloc_tile_pool(name="p3", bufs=1, side="left")
        att = tc.alloc_tile_pool(name="att", bufs=1, side="right")
        attnT = p3.tile([P, KC, N], BF16)

        def s_chunk(h, es_h, kt):
            pb = (h % 2) * HD
            qT = qkT[pb:pb + HD, h // 2, :]
            kT = qkT[pb:pb + HD, KC + h // 2, :]
            sps = psum.tile([P, 2, 512], F32, tag="sps", bufs=2,
                            name=f"s_{h}_{kt}")
            for j in range(2):
                nc.tensor.matmul(sps[:, j, :], kT[:, kt * P:(kt + 1) * P],
                                 qT[:, j * 512:(j + 1) * 512],
                                 start=True, stop=True)
            nc.scalar.activation(out=es_h[:, kt, :],
                                 in_=sps.rearrange("p a b -> p (a b)"),
                                 func=AF.Exp, scale=0.125)

        def av_chunk(h, es_h, avs, kt):
            for j in range(2):
                nc.tensor.matmul(avs[j], v_aug[:, kt, h, :],
                                 es_h[:, kt, j * 512:(j + 1) * 512],
                                 start=(kt == 0), stop=(kt == NT - 1))

        def norm_head(h, avs):
            pb = (h % 2) * HD
            # Softmax denominators: row HD holds sum_k exp(S). Copy it out on
            # its own lane (partition 64), bounce through DRAM into a
            # [128, 8] compact layout so the (slow, per-lane iterative)
            # reciprocal runs across all partitions, then broadcast back over
            # partitions 0..63 (step-0 partition reads are only legal from
            # DRAM).
            den = att.tile([VW, N], F32, tag="den", bufs=2, name=f"den{h}")
            for j in range(2):
                nc.vector.tensor_copy(out=den[HD:VW, j * 512:(j + 1) * 512],
                                      in_=avs[j][HD:VW, :])
            rcomp = att.tile([P, NT], F32, tag="rc", bufs=2, name=f"rc{h}")
            nc.sync.dma_start(out=rcomp, in_=den[HD:VW, :])
            nc.vector.reciprocal(out=rcomp, in_=rcomp)
            rd2 = dram.tile([1, N], F32, tag="rd2", bufs=3)
            nc.sync.dma_start(out=rd2[0, :].rearrange("(p m) -> p m", p=P),
                              in_=rcomp)
            rbc = att.tile([HD, N], F32, tag="rbc", bufs=2, name=f"rbc{h}")
            rd2a = rd2[0, :]
            rbc_src = bass.AP(tensor=rd2a.tensor, offset=rd2a.offset,
                              ap=[[0, HD]] + list(rd2a.ap))
            nc.sync.dma_start(out=rbc, in_=rbc_src)
            # Normalize straight out of PSUM. Even heads write attnT's
            # partitions 0..63 directly; odd heads bounce via SBUF + DMA
            # (DVE lanes cannot shift partition ranges).
            if pb == 0:
                for j in range(2):
                    nc.vector.tensor_mul(
                        out=attnT[0:HD, h // 2, j * 512:(j + 1) * 512],
                        in0=avs[j][0:HD, :], in1=rbc[:, j * 512:(j + 1) * 512])
            else:
                bounce = att.tile([HD, N], BF16, tag="bounce", bufs=2,
                                  name=f"bounce{h}")
                for j in range(2):
                    nc.vector.tensor_mul(out=bounce[:, j * 512:(j + 1) * 512],
                                         in0=avs[j][0:HD, :],
                                         in1=rbc[:, j * 512:(j + 1) * 512])
                nc.sync.dma_start(out=attnT[pb:pb + HD, h // 2, :], in_=bounce)

        # Per head: S/exp chunks stream per key tile with the head's own AV
        # chunks interleaved two key tiles behind — the PE alternates S and
        # AV work at fine grain so the ACT exp stream (the attention-phase
        # bottleneck) never starves. Pair-swapped order makes the final head
        # even, whose normalize writes attnT directly (shortest tail).
        order = [1, 0, 3, 2, 5, 4, 7, 6, 9, 8, 11, 10]
        for h in order:
            es_h = att.tile([P, NT, N], BF16, tag="es", bufs=2, name=f"es{h}")
            avs = [psum.tile([VW, 512], F32, tag="av", bufs=4,
                             name=f"av{j}_{h}") for j in range(2)]
            for kt in range(NT):
                s_chunk(h, es_h, kt)
                if kt >= 2:
                    av_chunk(h, es_h, avs, kt - 2)
            for kt in range(NT - 2, NT):
                av_chunk(h, es_h, avs, kt)
            norm_head(h, avs)

        att.release()
        p2.release()

        # ---------------------------------------------------------------
        # Phase 4: proj + residual, LN2 -> x2_lnT
        # ---------------------------------------------------------------
        # wfc2 isn't needed until phase 6; loading it here (into SBUF freed
        # by the attention pools) keeps the attention-phase footprint low.
        w2 = tc.alloc_tile_pool(name="w2", bufs=1, side="right")
        wfc2_sb = w2.tile([P, KH, C], BF16)
        for ko in range(KH):
            nc.sync.dma_start(out=wfc2_sb[:, ko, :],
                              in_=wfc2_d.ap()[ko * P:(ko + 1) * P, :])

        p4 = tc.alloc_tile_pool(name="p4", bufs=1, side="right")
        ln2 = tc.alloc_tile_pool(name="ln2", bufs=3, side="right")
        x2lnT = p4.tile([P, KC, N], BF16)

        for t in range(NT):
            ps = psum.tile([P, 2, 512], F32, tag="sps", bufs=2, name="ps_pr")
            for ko in range(KC):
                for j, (n0, nn) in enumerate(((0, 512), (512, 256))):
                    nc.tensor.matmul(ps[:, j, :nn],
                                     attnT[:, ko, t * P:(t + 1) * P],
                                     wproj_sb[:, ko, n0:n0 + nn],
                                     start=(ko == 0), stop=(ko == KC - 1))
            for j, (n0, nn) in enumerate(((0, 512), (512, 256))):
                xs = x_sb[:, t, n0:n0 + nn]
                nc.vector.tensor_add(out=xs, in0=xs, in1=ps[:, j, :nn])
                if "bproj" in g_beta:
                    nc.vector.tensor_add(out=xs, in0=xs,
                                         in1=g_beta["bproj"][:, n0:n0 + nn])
            xln = layernorm_tile(ln2, x_sb[:, t, :], g_beta.get("g2"),
                                 g_beta.get("beta2"), "ln2")
            transpose_to(xln, x2lnT, t)

        p3.release()

        # ---------------------------------------------------------------
        # Phase 5: fc1 + gelu -> h^T (feature-major bf16)
        # ---------------------------------------------------------------
        p5 = tc.alloc_tile_pool(name="p5", bufs=1, side="left")
        hT = p5.tile([P, KH, N], BF16)

        for m in range(KH):
            ps = psum.tile([P, 2, 512], F32, tag="sps", bufs=2, name="ps_f1")
            for ko in range(KC):
                for j in range(2):
                    nc.tensor.matmul(ps[:, j, :],
                                     wfc1_sb[:, ko, m * P:(m + 1) * P],
                                     x2lnT[:, ko, j * 512:(j + 1) * 512],
                                     start=(ko == 0), stop=(ko == KC - 1))
            bias = bfc1_sb[:, m:m + 1] if bfc1_sb is not None else 0.0
            for j in range(2):
                if not SIM_GELU:
                    nc.scalar.activation(out=hT[:, m, j * 512:(j + 1) * 512],
                                         in_=ps[:, j, :], func=AF.Gelu,
                                         bias=bias, scale=1.0)
                else:
                    a = ln2.tile([P, 512], F32, tag="g_a", bufs=2)
                    nc.scalar.activation(out=a, in_=ps[:, j, :], func=AF.Copy,
                                         bias=0.0, scale=1.0)
                    if bfc1_sb is not None:
                        nc.vector.tensor_scalar_add(a, a, bfc1_sb[:, m:m + 1])
                    u = ln2.tile([P, 512], F32, tag="g_u", bufs=2)
                    nc.vector.tensor_mul(out=u, in0=a, in1=a)
                    nc.vector.tensor_mul(out=u, in0=u, in1=a)
                    nc.vector.tensor_scalar_mul(u, u, 0.044715)
                    nc.vector.tensor_add(out=u, in0=u, in1=a)
                    nc.scalar.activation(out=u, in_=u, func=AF.Tanh,
                                         bias=0.0, scale=0.7978845608028654)
                    nc.vector.tensor_scalar_add(u, u, 1.0)
                    nc.vector.tensor_scalar_mul(a, a, 0.5)
                    nc.vector.tensor_mul(out=hT[:, m, j * 512:(j + 1) * 512],
                                         in0=a, in1=u)

        ln2.release()
        p4.release()

        # ---------------------------------------------------------------
        # Phase 6: fc2 + residual -> out
        # ---------------------------------------------------------------
        for t in range(NT):
            ps = psum.tile([P, 2, 512], F32, tag="sps", bufs=2, name="ps_f2")
            for ko in range(KH):
                for j, (n0, nn) in enumerate(((0, 512), (512, 256))):
                    nc.tensor.matmul(ps[:, j, :nn],
                                     hT[:, ko, t * P:(t + 1) * P],
                                     wfc2_sb[:, ko, n0:n0 + nn],
                                     start=(ko == 0), stop=(ko == KH - 1))
            for j, (n0, nn) in enumerate(((0, 512), (512, 256))):
                xs = x_sb[:, t, n0:n0 + nn]
                nc.vector.tensor_add(out=xs, in0=xs, in1=ps[:, j, :nn])
                if "bfc2" in g_beta:
                    nc.vector.tensor_add(out=xs, in0=xs,
                                         in1=g_beta["bfc2"][:, n0:n0 + nn])
            nc.sync.dma_start(out=out_d.ap()[t * P:(t + 1) * P, :],
                              in_=x_sb[:, t, :])

        w2.release()
        p5.release()
        wpre.release()
        persist.release()
        dram.release()
        psum.release()

    nc.compile()
    return nc


def _prep(inputs):
    """Host-side prep: shard x over B, cast weights to bf16, compute gates."""
    f = {k: np.asarray(v) for k, v in inputs.items()}
    bf = ml_dtypes.bfloat16

    flags = (
        bool(np.any(f["b_qkv"])),
        not np.all(f["g1"] == 1.0),
        bool(np.any(f["beta1"])),
        not np.all(f["g2"] == 1.0),
        bool(np.any(f["beta2"])),
        bool(np.any(f["b_fc1"])),
        bool(np.any(f["b_proj"])),
        bool(np.any(f["b_fc2"])),
    )
    (use_bqkv, use_g1, use_beta1, use_g2, use_beta2, use_bfc1, use_bproj,
     use_bfc2) = flags

    common = {
        "wqkv": np.ascontiguousarray(f["w_qkv"].astype(bf)),
        "wproj": np.ascontiguousarray(f["w_proj"].astype(bf)),
        "wfc1": np.ascontiguousarray(f["w_fc1"].astype(bf)),
        "wfc2": np.ascontiguousarray(f["w_fc2"].astype(bf)),
    }
    for name, key, use in (
        ("bqkv", "b_qkv", use_bqkv), ("g1", "g1", use_g1),
        ("beta1", "beta1", use_beta1), ("g2", "g2", use_g2),
        ("beta2", "beta2", use_beta2), ("bfc1", "b_fc1", use_bfc1),
        ("bproj", "b_proj", use_bproj), ("bfc2", "b_fc2", use_bfc2),
    ):
        if use:
            common[name] = np.ascontiguousarray(f[key].astype(np.float32))

    x = f["x"].astype(np.float32)
    in_maps = [dict(common, x=np.ascontiguousarray(x[i])) for i in range(B)]
    return flags, in_maps


LAST_RESULT = None


def kernel(**inputs):
    global LAST_RESULT
    flags, in_maps = _prep(inputs)
    if flags not in _cache:
        _cache[flags] = _build(flags)
    nc = _cache[flags]
    res = bass_utils.run_bass_kernel_spmd(nc, in_maps, core_ids=list(range(B)))
    LAST_RESULT = res
    out = np.stack([r["out"] for r in res.results], axis=0)
    return out.astype(np.float32)
